# revision 69
# baseline (speedup 1.0000x reference)
"""DeltaNet multi-scale kernel for TRN2, 8-core SPMD.

Sharding: core c handles batch b=c//4 and heads {2*(c%4), 2*(c%4)+1}.
Per 512-row block j of the sequence, a ReduceScatter over groups
[[0-3],[4-7]] sums the out-projection partials and hands core (b, r)
rows [512j+128r : 512j+128r+128] for residual+LayerNorm.

Math (per head h, scale kappa, chunk of C=128 steps):
  S_t = r_t * S_{t-1} + k_t v_t^T,  o_t = sum_k w_tk * q_t^T S_tk
  With L_t = cumsum(log r) (chunk-local):
  o_t = exp(Lm_t) * [ (ATd^T V)_t + (q_t S_0) ] / sumexp(mix)_t
    where ATd[s,t] = (k_s . q_t) * exp(-L_s) * 1[s<=t], Lm = L + mix
  S_new = exp(L_C) * (S_0 + (K*exp(-L_s))^T V)

V3: bf16 inner matmuls keyed on the moving operand, 4-scale batched
[128,512] PSUM tiles for o and S, single activation table
(log-free cumprod decay tables), k projected once (transposed) and
recovered natural via PE transpose, 1-chunk software pipeline with the
q/k projections spread one piece per iteration.
"""

import math
from contextlib import ExitStack

import concourse.bass as bass
from concourse import mybir
from concourse.alu_op_type import AluOpType
from bass_rust import AxisListType

F32 = mybir.dt.float32
F32R = mybir.dt.float32r
BF16 = mybir.dt.bfloat16

C = 128          # chunk length
NCH = 16         # chunks (T=2048)
SUP = 4          # chunks per super-chunk
NSUP = NCH // SUP
D = 128          # head dim
H2 = 2           # local heads per core
K = 4            # decay scales
T = 2048
HID = 1024
NKC = 8          # contraction tiles over HID
CLAMP_MAX = 0.999995
NCLAMP = -math.log(CLAMP_MAX)   # = -log r clamp floor (positive, tiny)
LN_EPS = 1e-5


def r32(ap):
    return ap.bitcast(F32R)


def build(ctx: ExitStack, tc, ins: dict, outs: dict, single_core: bool = False):
    nc = tc.nc
    ACT = mybir.ActivationFunctionType
    xbT_d = ins["xbT"]          # [1024, 2048] bf16
    wqk_d = ins["wqk"]          # [1024, 512] bf16 (q 256 | k 256, transposed proj)
    wv_d = ins["wv"]            # [1024, 256] bf16
    wbm_d = ins["wbm16"]        # [1024, 40] bf16 (beta 0:8 | dead | mix 32:40)
    bbm_d = ins["bbm16"]        # [1, 40] bf16
    lbr8_d = ins["lbr8"]        # [8, 1] f32 log(base_r)
    wo_d = ins["wo"]            # [256, 1024] bf16
    mask_d = ins["mask01"]      # [128, 128] f32 upper-tri (s<=t)
    ident_d = ins["ident"]      # [128, 128] f32
    ident16_d = ins["ident16"]  # [128, 128] bf16
    lng_d, lnb_d = ins["ln_g16"], ins["ln_b16"]   # [1024] bf16
    xres_d = ins["x_res"]       # [512, 1024] f32 rows {512j+128r}, bo pre-added
    out_d = outs["out"]         # [512, 1024] f32 row-block j = LN of rs block j

    consts = ctx.enter_context(tc.tile_pool(name="consts", bufs=1))
    xpool = ctx.enter_context(tc.tile_pool(name="xbt", bufs=1))
    wpool = ctx.enter_context(tc.tile_pool(name="weights", bufs=1))
    a2 = ctx.enter_context(tc.tile_pool(name="phasea2", bufs=2))
    tabs = ctx.enter_context(tc.tile_pool(name="tabs", bufs=4))
    spool = ctx.enter_context(tc.tile_pool(name="states", bufs=1))
    qk2 = ctx.enter_context(tc.tile_pool(name="qk2", bufs=2))
    work = ctx.enter_context(tc.tile_pool(name="work", bufs=2))
    kd16 = ctx.enter_context(tc.tile_pool(name="kd16", bufs=2))
    dram = ctx.enter_context(tc.tile_pool(name="dram", bufs=1, space="DRAM"))

    ps_big = ctx.enter_context(tc.tile_pool(name="psbig", bufs=2, space="PSUM"))
    ps_o = ctx.enter_context(tc.tile_pool(name="pso", bufs=2, space="PSUM"))
    ps_s = ctx.enter_context(tc.tile_pool(name="pss", bufs=2, space="PSUM"))
    ps_at = ctx.enter_context(tc.tile_pool(name="psat", bufs=2, space="PSUM"))

    # ---- small constants (scalar queue; bulk loads go on sync) ----
    invbr8 = consts.tile([8, 1], F32)
    nc.scalar.dma_start(out=invbr8, in_=lbr8_d)   # input now carries 1/base_r
    bbm = consts.tile([1, 40], BF16)
    nc.scalar.dma_start(out=bbm, in_=bbm_d)
    ones512 = consts.tile([1, 512], BF16)
    nc.scalar.dma_start(out=ones512, in_=ins["ones512"])

    mask2 = consts.tile([128, 2, 128], F32)
    nc.scalar.dma_start(out=mask2[:, 0, :], in_=mask_d)
    nc.scalar.dma_start(out=mask2[:, 1, :], in_=mask_d)
    ident = consts.tile([128, 128], F32)
    nc.scalar.dma_start(out=ident, in_=ident_d)
    ident16 = consts.tile([128, 128], BF16)
    nc.scalar.dma_start(out=ident16, in_=ident16_d)
    eps_t = consts.tile([128, 1], F32)
    nc.vector.memset(eps_t, LN_EPS)

    def row_bcast(dst, src, n=128, q=None):
        src_b = bass.AP(tensor=src.tensor, offset=src.offset, ap=[[0, n]] + list(src.ap))
        (q or nc.sync).dma_start(out=dst, in_=src_b)

    # xbT: one [128, NKC, 2048] bf16 tile loaded super-chunk-major, one big
    # DMA per slab (the ~650ns per-DMA HWDGE overhead dominated startup when
    # each (j, slab) pair was its own transfer). Weights are single DMAs on
    # the scalar queue.
    xbT = xpool.tile([128, NKC, T], BF16, name="xbT")
    xbT_r = xbT_d.rearrange("(j p) t -> p j t", p=128)
    wqk = wpool.tile([128, NKC, 512], BF16)
    wv = wpool.tile([128, NKC, 256], BF16)
    wbm = wpool.tile([128, NKC, 40], BF16)
    wo = wpool.tile([128, 2, HID], BF16)
    nc.sync.dma_start(out=xbT[:, :, 0:512], in_=xbT_r[:, :, 0:512])
    nc.sync.dma_start(out=wbm, in_=wbm_d.rearrange("(j p) d -> p j d", p=128))
    nc.sync.dma_start(out=wqk, in_=wqk_d.rearrange("(j p) d -> p j d", p=128))
    nc.scalar.dma_start(out=wv, in_=wv_d.rearrange("(j p) d -> p j d", p=128))
    for sc in range(1, NSUP):
        ssl = slice(512 * sc, 512 * (sc + 1))
        nc.sync.dma_start(out=xbT[:, :, ssl], in_=xbT_r[:, :, ssl])
    nc.scalar.dma_start(out=wo, in_=wo_d.rearrange("(m p) n -> p m n", p=128))
    wo0, wo1 = wo[:, 0, :], wo[:, 1, :]

    g_bc = consts.tile([128, HID], BF16)
    row_bcast(g_bc, lng_d, q=nc.scalar)
    b_bc = consts.tile([128, HID], BF16)
    row_bcast(b_bc, lnb_d, q=nc.scalar)

    # ---- Phase A (per super-chunk): beta/mix -> decay tables, log-free ----
    # The decay cumulation is a cumPROD of 1/r_eff (range-checked: max ~7e5),
    # so only Exp/Copy activations are needed and one act table serves the
    # whole kernel. All work stays on-chip in the natural [8, 512] layout
    # (4 chunk-segmented scans), with tiny [8,128]->[128,8] PE transposes.
    # tables[sc] = (expNegLT, ewnT, PCrep, decayKT), each [128, 32], column
    # col = 8*(c%4) + 4*h + kappa.  Called interleaved with the chunk loop.
    tables = [None] * NSUP

    def phase_a(sc):
        nsl = slice(512 * sc, 512 * (sc + 1))
        bm_big = ps_big.tile([128, 512], F32, tag="big")
        bA = bm_big[0:8, :]
        for j in range(NKC):
            nc.tensor.matmul(bA, lhsT=wbm[:, j, 0:8], rhs=xbT[:, j, nsl],
                             start=(j == 0), stop=False, skip_group_check=True)
        nc.tensor.matmul(bA, lhsT=bbm[:, 0:8], rhs=ones512, start=False,
                         stop=True, skip_group_check=True)
        bm_big2 = ps_at.tile([128, 256], F32, tag="atp")
        bB0 = bm_big2[0:8, 0:256]
        for j in range(NKC):
            nc.tensor.matmul(bB0, lhsT=wbm[:, j, 32:40], rhs=xbT[:, j, nsl][:, 0:256],
                             start=(j == 0), stop=False, skip_group_check=True)
        nc.tensor.matmul(bB0, lhsT=bbm[:, 32:40], rhs=ones512[:, 0:256],
                         start=False, stop=True, skip_group_check=True)
        # second half of mix (cols 256:512) in another tile
        bm_big3 = ps_at.tile([128, 256], F32, tag="atp")
        bC = bm_big3[0:8, :]
        for j in range(NKC):
            nc.tensor.matmul(bC, lhsT=wbm[:, j, 32:40], rhs=xbT[:, j, nsl][:, 256:512],
                             start=(j == 0), stop=False, skip_group_check=True)
        nc.tensor.matmul(bC, lhsT=bbm[:, 32:40], rhs=ones512[:, 0:256],
                         start=False, stop=True, skip_group_check=True)
        e8 = a2.tile([8, 512], F32, tag="e8", name="e8")
        nc.scalar.activation(e8, bA, ACT.Exp, scale=-1.0)
        expmix8 = a2.tile([8, 512], F32, tag="expmix8", name="expmix8")
        nc.scalar.activation(expmix8[:, 0:256], bB0, ACT.Exp)
        nc.scalar.activation(expmix8[:, 256:512], bC, ACT.Exp)
        # 1/r_eff = (1 + e^{-z}) / base_r    (CLAMP_MAX can never bind here)
        invr8 = a2.tile([8, 512], F32, tag="invr8", name="invr8")
        nc.vector.tensor_scalar(out=invr8, in0=e8, scalar1=1.0,
                                scalar2=invbr8,
                                op0=AluOpType.add, op1=AluOpType.mult)
        PL8 = a2.tile([8, 512], F32, tag="PL8", name="PL8")  # e^{-L}, per chunk
        for cc in range(SUP):
            tsl = slice(128 * cc, 128 * (cc + 1))
            nc.vector.tensor_tensor_scan(out=PL8[:, tsl], data0=invr8[:, tsl],
                                         data1=invr8[:, tsl], initial=1.0,
                                         op0=AluOpType.mult, op1=AluOpType.bypass)

        id8 = ident[0:8, 0:8]
        tp = ps_at.tile([128, 256], F32, tag="atp")
        for cc in range(SUP):
            tsl = slice(128 * cc, 128 * (cc + 1))
            nc.tensor.transpose(tp[:, 8 * cc:8 * (cc + 1)], PL8[:, tsl], id8)
            nc.tensor.transpose(tp[:, 64 + 8 * cc:64 + 8 * (cc + 1)],
                                expmix8[:, tsl], id8)
        expNegLT = tabs.tile([128, 32], F32, tag="expNegLT", name="expNegLT")
        nc.scalar.activation(expNegLT, tp[:, 0:32], ACT.Copy)
        expmixT = tabs.tile([128, 32], F32, tag="expmixT", name="expmixT")
        nc.scalar.activation(expmixT, tp[:, 64:96], ACT.Copy)
        recipT = a2.tile([128, 32], F32, tag="recipT", name="recipT")  # e^{L}
        nc.vector.reciprocal(recipT, expNegLT)
        ewmT = tabs.tile([128, 32], F32, tag="ewmT", name="ewmT")   # e^{L+mix}
        nc.vector.tensor_tensor(out=ewmT, in0=expmixT, in1=recipT, op=AluOpType.mult)

        # PC: e^{L_C} = 1/PL8 at each chunk end (partition-base-0 access),
        # bounced through DRAM to reshape [8,4] -> [1,32] then broadcast
        rpc8 = a2.tile([8, 4, 1], F32, tag="rpc8", name="rpc8")
        nc.vector.reciprocal(rpc8, PL8.rearrange("p (c t) -> p c t", c=4)[:, :, 127:128])
        pc_d = dram.tile([1, 32], F32, name=f"pc_d{sc}")
        nc.gpsimd.dma_start(out=pc_d, in_=rpc8)          # col' = 4*hk + c
        pc_d2 = dram.tile([1, 32], F32, name=f"pc_d2{sc}")
        pc_src = bass.AP(tensor=pc_d.tensor, offset=pc_d.offset,
                         ap=[[1, 4], [4, 8]])
        nc.gpsimd.dma_start(out=pc_d2, in_=pc_src)       # col = 8*c + hk
        PCrep = tabs.tile([128, 32], F32, tag="PCrep", name="PCrep")
        row_bcast(PCrep, pc_d2[0, :], q=nc.gpsimd)

        seT = tabs.tile([128, 8], F32, tag="seT", name="seT")
        nc.vector.reduce_sum(out=seT, in_=expmixT.rearrange("p (c k) -> p c k", k=4),
                             axis=AxisListType.X)
        rwT = tabs.tile([128, 8], F32, tag="rwT", name="rwT")
        nc.vector.reciprocal(rwT, seT)
        ewnT = tabs.tile([128, 32], F32, tag="ewnT", name="ewnT")
        ewmT4 = ewmT.rearrange("p (c k) -> p c k", k=4)
        ewnT4 = ewnT.rearrange("p (c k) -> p c k", k=4)
        for kk in range(K):
            nc.vector.tensor_tensor(out=ewnT4[:, :, kk], in0=ewmT4[:, :, kk], in1=rwT,
                                    op=AluOpType.mult)
        decayKT = tabs.tile([128, 32], F32, tag="decayKT", name="decayKT")
        nc.vector.tensor_tensor(out=decayKT, in0=expNegLT, in1=PCrep, op=AluOpType.mult)
        tables[sc] = (expNegLT, ewnT, PCrep, decayKT)

    phase_a(0)

    # ---- states: per head one [128, 4*128] f32 tile (4 scales contiguous) ----
    Sa = [spool.tile([128, K * D], BF16, tag=f"Sa{h}", name=f"Sa{h}") for h in range(H2)]
    Sb = [spool.tile([128, K * D], BF16, tag=f"Sb{h}", name=f"Sb{h}") for h in range(H2)]
    for s in Sa:
        nc.vector.memset(s, 0.0)

    partial = dram.tile([T, HID], F32)
    rs_blks = [dram.tile([128, HID], F32, name=f"rs_blk{j}") for j in range(NSUP)]

    def ln_block(j):
        """residual + LayerNorm for rs block j -> out rows [128j:128j+128]."""
        rsl = slice(128 * j, 128 * (j + 1))
        yt = work.tile([128, HID], F32, tag="lnyt", name="yt")
        nc.sync.dma_start(out=yt, in_=rs_blks[j][:])
        xr = work.tile([128, HID], F32, tag="lnxr", name="xr")
        nc.sync.dma_start(out=xr, in_=xres_d[rsl, :])
        yb = work.tile([128, HID], F32, tag="lnyb", name="yb")
        nc.vector.tensor_tensor(out=yb, in0=yt, in1=xr, op=AluOpType.add)
        BSD = nc.vector.BN_STATS_DIM
        stats = work.tile([128, 2, BSD], F32, tag="stats")
        yb3 = yb.rearrange("p (s d) -> p s d", s=2)
        for s in range(2):
            nc.vector.bn_stats(out=stats[:, s, :], in_=yb3[:, s, :])
        mv = work.tile([128, 2], F32, tag="mv")
        nc.vector.bn_aggr(out=mv, in_=stats)
        sd = work.tile([128, 1], F32, tag="lnsd")
        nc.scalar.activation(sd, mv[:, 1:2], ACT.Sqrt, bias=eps_t)
        rstd = work.tile([128, 1], F32, tag="rstd")
        nc.vector.reciprocal(rstd, sd)
        yn = work.tile([128, HID], BF16, tag="lnyn", name="yn")
        nc.vector.tensor_scalar(out=yn, in0=yb, scalar1=mv[:, 0:1], scalar2=rstd,
                                op0=AluOpType.subtract, op1=AluOpType.mult)
        yg = work.tile([128, HID], BF16, tag="lnyg", name="yg")
        nc.vector.tensor_tensor(out=yg, in0=yn, in1=g_bc, op=AluOpType.mult)
        yo = work.tile([128, HID], F32, tag="lnyo", name="yo")
        nc.vector.tensor_tensor(out=yo, in0=yg, in1=b_bc, op=AluOpType.add)
        nc.sync.dma_start(out=out_d[rsl, :], in_=yo)

    # ---- Phase B: software-pipelined chunk loop ----
    # prep(c): everything for chunk c that does not depend on states:
    #   (at sc boundary) q/k transposed projections + elu
    #   v projection + vn, kn via PE transpose of kTs, Kd, AT, ATm, ATd
    # main(c): o-psum (qS batched + 4x ATd^T V), acc -> ho -> hoT, out-proj,
    #   dsp-psum (4x Kd^T V), state update, LN at block end.
    qkts = [None] * NSUP   # (qTs, kTs) per super-chunk
    vns = [None] * NCH
    kns = [None] * NCH
    ATds = [None] * NCH
    Kds = [None] * NCH

    def qk_piece(sc, i):
        """One of the four (q|k, head) projection pieces for super-chunk sc;
        spread one per iteration so the PE burst doesn't starve DVE/ACT."""
        if i == 0:
            qTs = qk2.tile([128, H2, 512], BF16, tag="qTs")
            kTs = qk2.tile([128, H2, 512], BF16, tag="kTs")
            qkts[sc] = (qTs, kTs)
        qTs, kTs = qkts[sc]
        wc, m, dst = ((256, 0, kTs), (256, 1, kTs), (0, 0, qTs), (0, 1, qTs))[i]
        ssl = slice(512 * sc, 512 * (sc + 1))
        pp_t = ps_big.tile([128, 512], F32, tag="big")
        for j in range(NKC):
            nc.tensor.matmul(pp_t, lhsT=wqk[:, j, wc + 128 * m:wc + 128 * (m + 1)],
                             rhs=xbT[:, j, ssl],
                             start=(j == 0), stop=(j == NKC - 1))
        rq = work.tile([128, 512], BF16, tag="rq")
        eq = work.tile([128, 512], BF16, tag="eq")
        nc.scalar.activation(rq, pp_t, ACT.Relu)
        nc.scalar.activation(eq, pp_t, ACT.Exp)
        # elu1 = relu(z) + min(exp(z), 1)
        nc.vector.scalar_tensor_tensor(out=dst[:, m, :], in0=eq, scalar=1.0,
                                       in1=rq, op0=AluOpType.min,
                                       op1=AluOpType.add)

    at2s = [None] * NCH

    def prep_pe(c):
        """PE-side production for chunk c: v projection, kn transposes, AT."""
        sc = c // SUP
        qTs, kTs = qkts[sc]
        csl = slice(C * c, C * (c + 1))
        co = C * (c % SUP)

        # v natural for this chunk; kn transposes share the same PSUM bank
        vpbig = ps_big.tile([128, 512], F32, tag="big")
        vp = vpbig[:, 0:256]
        for j in range(NKC):
            nc.tensor.matmul(vp, lhsT=xbT[:, j, csl], rhs=wv[:, j, :],
                             start=(j == 0), stop=(j == NKC - 1),
                             skip_group_check=True)
        vn = work.tile([128, 256], BF16, tag="vn")
        nc.scalar.activation(vn, vp, ACT.Copy)
        vns[c] = vn

        # kn (natural, elu'd, bf16) via PE transpose of kTs chunk slices
        ktp = vpbig[:, 256:384].bitcast(BF16)   # [128, 256] bf16 view
        for h in range(H2):
            nc.tensor.transpose(ktp[:, 128 * h:128 * (h + 1)],
                                kTs[:, h, co:co + C], ident16)
        kn = work.tile([128, 256], BF16, tag="kn")
        nc.scalar.activation(kn, ktp, ACT.Copy)
        kns[c] = kn

        # AT for both heads in one PSUM tile
        at2 = ps_at.tile([128, 256], F32, tag="atp")
        for h in range(H2):
            nc.tensor.matmul(at2[:, 128 * h:128 * (h + 1)],
                             lhsT=kTs[:, h, co:co + C], rhs=qTs[:, h, co:co + C],
                             start=True, stop=True, skip_group_check=True)
        at2s[c] = at2

    def prep_rest(c):
        """DVE/Pool-side production for chunk c: ATm -> ATd, Kd."""
        sc = c // SUP
        expNegLT, ewnT, PCrep, decayKT = tables[sc]
        kn, at2 = kns[c], at2s[c]
        ATd = kd16.tile([128, H2, K, 128], BF16, tag="ATd")
        Kd = kd16.tile([128, H2, K, 128], BF16, tag="Kd")
        ATm = work.tile([128, 2, 128], F32, tag="ATm")
        nc.vector.tensor_tensor(out=ATm.rearrange("p h d -> p (h d)"), in0=at2,
                                in1=mask2.rearrange("p h d -> p (h d)"),
                                op=AluOpType.mult)
        for h in range(H2):
            for kk in range(K):
                col = 8 * (c % SUP) + 4 * h + kk
                nc.gpsimd.tensor_scalar_mul(ATd[:, h, kk, :], ATm[:, h, :],
                                            expNegLT[:, col:col + 1])
                nc.vector.tensor_scalar_mul(Kd[:, h, kk, :],
                                            kn[:, 128 * h:128 * (h + 1)],
                                            decayKT[:, col:col + 1])
        ATds[c] = ATd
        Kds[c] = Kd

    for i in range(4):
        qk_piece(0, i)
    prep_pe(0)
    prep_rest(0)
    Scur, Snxt = Sa, Sb
    for c in range(NCH):
        sc = c // SUP
        qTs, kTs = qkts[sc]
        expNegLT, ewnT, PCrep, decayKT = tables[sc]
        csl = slice(C * c, C * (c + 1))
        co = C * (c % SUP)
        vn, kn, ATd, Kd = vns[c], kns[c], ATds[c], Kds[c]

        # -- 1. dsp matmuls (inputs all ready from prep(c)) so the DVE state
        # update can start immediately, then the o matmuls, then prep(c+1)
        # fills PE while DVE/ACT chew on the chunk's elementwise chain.
        s_pss, o_pss = [], []
        for h in range(H2):
            vslice = vn[:, 128 * h:128 * (h + 1)]
            s_ps = ps_s.tile([128, 512], F32, tag="s")
            for kk in range(K):
                nc.tensor.matmul(s_ps[:, 128 * kk:128 * (kk + 1)],
                                 lhsT=Kd[:, h, kk, :], rhs=vslice,
                                 start=True, stop=True, skip_group_check=True)
            s_pss.append(s_ps)
        # -- 3. o-psum: batched q @ S4 (f32r moving) + per-scale ATd^T V (bf16)
        for h in range(H2):
            vslice = vn[:, 128 * h:128 * (h + 1)]
            o_ps = ps_o.tile([128, 512], F32, tag="o")
            if c > 0:
                nc.tensor.matmul(o_ps, lhsT=qTs[:, h, co:co + C], rhs=Scur[h][:],
                                 start=True, stop=False, skip_group_check=True)
            for kk in range(K):
                nc.tensor.matmul(o_ps[:, 128 * kk:128 * (kk + 1)],
                                 lhsT=ATd[:, h, kk, :], rhs=vslice,
                                 start=(c == 0), stop=True, skip_group_check=True)
            o_pss.append(o_ps)

        # -- 2. state update S' = PC*S + dsp (DVE long pole, issued early)
        for h in range(H2):
            for kk in range(K):
                col = 8 * (c % SUP) + 4 * h + kk
                ksl = slice(128 * kk, 128 * (kk + 1))
                nc.vector.scalar_tensor_tensor(out=Snxt[h][:, ksl],
                                               in0=Scur[h][:, ksl],
                                               scalar=PCrep[:, col:col + 1],
                                               in1=s_pss[h][:, ksl],
                                               op0=AluOpType.mult, op1=AluOpType.add)
        # -- 4. next chunk's PE-side production (overlaps DVE/ACT work below);
        # the next super-chunk's decay tables build mid-super, well ahead of
        # first use
        if c // SUP + 1 < NSUP:
            qk_piece(c // SUP + 1, c % SUP)
        if c + 1 < NCH:
            prep_pe(c + 1)
        if c % SUP == 1 and c // SUP + 1 < NSUP:
            phase_a(c // SUP + 1)

        # -- 5. weighted sum over scales: k0,k1 on DVE, k2,k3 via ACT copies
        hoT = work.tile([128, 256], BF16, tag="hoT")
        tph = ps_at.tile([128, 256], F32, tag="atp")
        for h in range(H2):
            o_ps = o_pss[h]
            cb = 8 * (c % SUP) + 4 * h
            t0 = work.tile([128, 128], BF16, tag="accA")
            nc.vector.tensor_scalar_mul(t0, o_ps[:, 0:128], ewnT[:, cb:cb + 1])
            t1 = work.tile([128, 128], BF16, tag="accB")
            nc.vector.scalar_tensor_tensor(out=t1, in0=o_ps[:, 128:256],
                                           scalar=ewnT[:, cb + 1:cb + 2], in1=t0,
                                           op0=AluOpType.mult, op1=AluOpType.add)
            t2 = work.tile([128, 128], BF16, tag="accC")
            nc.vector.scalar_tensor_tensor(out=t2, in0=o_ps[:, 256:384],
                                           scalar=ewnT[:, cb + 2:cb + 3], in1=t1,
                                           op0=AluOpType.mult, op1=AluOpType.add)
            ho = work.tile([128, 128], BF16, tag="accF")
            nc.vector.scalar_tensor_tensor(out=ho, in0=o_ps[:, 384:512],
                                           scalar=ewnT[:, cb + 3:cb + 4], in1=t2,
                                           op0=AluOpType.mult, op1=AluOpType.add)
            hoT_ps = tph[:, 64 * h:64 * (h + 1)].bitcast(BF16)
            nc.tensor.transpose(hoT_ps, ho, ident16)
        nc.scalar.activation(hoT, tph[:, 0:128].bitcast(BF16), ACT.Copy)

        for n in range(2):
            nsl = slice(512 * n, 512 * (n + 1))
            opp = ps_big.tile([128, 512], F32, tag="big")
            nc.tensor.matmul(opp, lhsT=hoT[:, 0:128], rhs=wo0[:, nsl],
                             start=True, stop=False)
            nc.tensor.matmul(opp, lhsT=hoT[:, 128:256], rhs=wo1[:, nsl],
                             start=False, stop=True)
            out_sb = work.tile([128, 512], F32, tag="outsb")
            nc.scalar.activation(out_sb, opp, ACT.Copy)
            nc.sync.dma_start(out=partial[csl, nsl], in_=out_sb)

        # -- 6. next chunk's DVE/Pool-side production
        if c + 1 < NCH:
            prep_rest(c + 1)

        Scur, Snxt = Snxt, Scur

        # output block j depends only on chunks 4j..4j+3
        if c % SUP == SUP - 1:
            j = c // SUP
            blk = slice(512 * j, 512 * (j + 1))
            if single_core:
                nc.sync.dma_start(out=rs_blks[j][:], in_=partial[512 * j:512 * j + 128, :])
            else:
                nc.gpsimd.collective_compute(
                    "ReduceScatter", AluOpType.add,
                    replica_groups=[[0, 1, 2, 3], [4, 5, 6, 7]],
                    ins=[partial[blk, :].opt()], outs=[rs_blks[j][:].opt()],
                )
            ln_block(j)


def make_in_maps(inputs: dict) -> list[dict]:
    """Build the 8 per-core input dicts from the full problem inputs."""
    import numpy as np
    import ml_dtypes
    BF = ml_dtypes.bfloat16
    x = np.asarray(inputs["x"], np.float32)
    Wq, Wk, Wv = (np.asarray(inputs[k], np.float32) for k in ("Wq", "Wk", "Wv"))
    Wb, bb = np.asarray(inputs["Wb"], np.float32), np.asarray(inputs["bb"], np.float32)
    Wm, bm = np.asarray(inputs["Wm"], np.float32), np.asarray(inputs["bm"], np.float32)
    Wo, bo = np.asarray(inputs["Wo"], np.float32), np.asarray(inputs["bo"], np.float32)
    base_logit = np.asarray(inputs["base_logit"], np.float32)
    ln_g, ln_b = np.asarray(inputs["ln_g"], np.float32), np.asarray(inputs["ln_b"], np.float32)
    base_r = 1.0 / (1.0 + np.exp(-base_logit.astype(np.float64)))  # (H, K)
    lbr = (1.0 / base_r).astype(np.float32)   # kernel wants 1/base_r

    mask01 = np.triu(np.ones((128, 128), np.float32))
    ident = np.eye(128, dtype=np.float32)
    ident16 = np.eye(128, dtype=BF)

    in_maps = []
    for c in range(8):
        b, r = c // 4, c % 4
        h0 = 2 * r
        cols = slice(128 * h0, 128 * (h0 + 2))
        wbm16 = np.zeros((1024, 40), np.float32)
        wbm16[:, 0:4] = np.repeat(Wb[:, h0:h0 + 1], 4, axis=1)
        wbm16[:, 4:8] = np.repeat(Wb[:, h0 + 1:h0 + 2], 4, axis=1)
        wbm16[:, 32:36] = Wm[:, 4 * h0:4 * (h0 + 1)]
        wbm16[:, 36:40] = Wm[:, 4 * (h0 + 1):4 * (h0 + 2)]
        bbm16 = np.zeros((1, 40), np.float32)
        bbm16[0, 0:4] = bb[h0]
        bbm16[0, 4:8] = bb[h0 + 1]
        bbm16[0, 32:36] = bm[4 * h0:4 * (h0 + 1)]
        bbm16[0, 36:40] = bm[4 * (h0 + 1):4 * (h0 + 2)]
        lbr8 = np.concatenate([lbr[h0], lbr[h0 + 1]])[:, None]
        rows = np.concatenate([np.arange(512 * j + 128 * r, 512 * j + 128 * r + 128)
                               for j in range(4)])
        in_maps.append(dict(
            xbT=np.ascontiguousarray(x[b].T).astype(BF),
            wqk=np.ascontiguousarray(
                np.concatenate([Wq[:, cols], Wk[:, cols]], axis=1)).astype(BF),
            wv=np.ascontiguousarray(Wv[:, cols]).astype(BF),
            wbm16=np.ascontiguousarray(wbm16).astype(BF),
            bbm16=np.ascontiguousarray(bbm16).astype(BF),
            lbr8=np.ascontiguousarray(lbr8),
            wo=np.ascontiguousarray(Wo[cols, :]).astype(BF),
            mask01=mask01,
            ident=ident,
            ident16=ident16,
            ones512=np.ones((1, 512), BF),
            ln_g16=ln_g.astype(BF), ln_b16=ln_b.astype(BF),
            x_res=np.ascontiguousarray(x[b, rows, :] + bo[None, :]),
        ))
    return in_maps


def assemble(core_outs: list) -> "np.ndarray":
    import numpy as np
    out = np.zeros((2, T, HID), np.float32)
    for c in range(8):
        b, r = c // 4, c % 4
        for j in range(4):
            out[b, 512 * j + 128 * r:512 * j + 128 * r + 128, :] = \
                core_outs[c][128 * j:128 * (j + 1)]
    return out

# ======================================================================
# Host entry point
# ======================================================================

_CACHE = {}

INPUT_SPECS = dict(
    xbT=((1024, 2048), BF16), wqk=((1024, 512), BF16), wv=((1024, 256), BF16),
    wbm16=((1024, 40), BF16), bbm16=((1, 40), BF16), lbr8=((8, 1), F32),
    wo=((256, 1024), BF16), mask01=((128, 128), F32), ident=((128, 128), F32),
    ident16=((128, 128), BF16), ones512=((1, 512), BF16),
    ln_g16=((1024,), BF16), ln_b16=((1024,), BF16), x_res=((512, 1024), F32),
)


def _build_program():
    if "nc" in _CACHE:
        return _CACHE["nc"]
    import concourse.bacc as bacc
    import concourse.tile as tile

    nc = bacc.Bacc("TRN2", target_bir_lowering=False, debug=False, num_devices=8)
    ins = {k: nc.dram_tensor(k, list(shape), dt, kind="ExternalInput").ap()
           for k, (shape, dt) in INPUT_SPECS.items()}
    outs = {"out": nc.dram_tensor("out", [512, HID], F32, kind="ExternalOutput").ap()}
    with tile.TileContext(nc) as tc:
        with ExitStack() as _ctx:
            build(_ctx, tc, ins, outs)
    nc.compile()
    _CACHE["nc"] = nc
    return nc


def kernel(**inputs):
    import numpy as np
    from concourse.bass_utils import run_bass_kernel_spmd

    nc = _build_program()
    in_maps = make_in_maps(inputs)
    res = run_bass_kernel_spmd(nc, in_maps, core_ids=list(range(8)))
    out = assemble([res.results[c]["out"] for c in range(8)])
    return out.astype(np.float32)
